# revision 1
# baseline (speedup 1.0000x reference)
"""DPLR SSM block kernel for Trainium2, 8 NeuronCores.

Math:  out = h @ (diag(a_diag) + p q^T).T + x @ b_mat          (B=64, H=8192, R=4)
           = h * a_diag  +  (h @ q) @ p^T  +  x @ b_mat

The dense (H,H) DPLR matrix is never materialized.  The memory-bound part is
streaming b_mat (256 MB fp32-worth of data).  Sharding: b_mat columns (= output
features) are split 8 ways; each core computes out[:, c*1024:(c+1)*1024].
x / q are replicated; host pre-permutes everything into the k-on-partitions
chunk layout the tensor engine wants, so no on-device transposes are needed.

fp32 matmul streams at 4 cycles/row on the PE, which would make the tensor
engine the critical path (~110us/core) over the DMA stream (~100us/core).
Instead x and b are carried as bf16 (hi, lo) pairs -- b ~= bh + bl with
bh = bf16(b), bl = bf16(b - bh) -- and the product uses three full-rate bf16
matmuls accumulating in fp32 PSUM:
    x @ b ~= xh@bh + xl@bh + xh@bl     (measured error ~4.6e-6, fp32-grade)
dropping only the xl@bl term (~2^-18 relative).  HBM traffic is unchanged
(2+2 bytes/element), but PE time drops to ~85us, restoring the DMA roofline.

Measured on trn2 (8 cores, looped-NEFF slope timing): ~119 us/core;
TimelineSim cost model predicts 117.8 us.  Idealized DMA roofline for the
36.6 MB/core stream at 368 GB/s is ~100 us.

Per core c (j0 = c*1024):
  hqT (4, 64)       = sum_ko  q[ko]^T(4x128) . hT[ko](128x64)          [PE fp32]
  ps  (64, 1024)    = 3-pass split-bf16 x @ b_slice                    [PE bf16]
                    + hqT^T(64x4) . pT(4x1024)                         [PE fp32]
  out (64, 1024)    = h_slice * a_slice  +  ps                         [DVE]
"""

import ml_dtypes
import numpy as np

import concourse.bass as bass
import concourse.mybir as mybir
from concourse import bacc
from concourse.bass_utils import run_bass_kernel_spmd
from concourse.tile import TileContext

H = 8192
R = 4
B = 64
NCORES = 8
JS = H // NCORES  # 1024 output columns per core
P = 128
KO = H // P  # 64 k-chunks
KT = 4  # k-chunks per DMA tile (tile = 128 x 4 x 2 x 1024 bf16 = 2 MB)
NT = KO // KT  # 16 b-mat DMA tiles per core

F32 = mybir.dt.float32
BF16 = mybir.dt.bfloat16
BF = ml_dtypes.bfloat16


def _build_nc(
    tiles: list[int] | None = None,
    bufs: int = 6,
    hq_tiles: tuple[int, int] = (4, 8),
    rank4_tile: int = 9,
    loop_n: int | None = None,
    aux_in_loop: bool = False,
    psum_split4: bool = False,
) -> bass.Bass:
    nc = bacc.Bacc("TRN2", target_bir_lowering=False, debug=False, num_devices=NCORES)

    xh = nc.dram_tensor("xh", (P, KO, B), BF16, kind="ExternalInput")
    xl = nc.dram_tensor("xl", (P, KO, B), BF16, kind="ExternalInput")
    ht = nc.dram_tensor("ht", (P, KO, B), F32, kind="ExternalInput")
    qk = nc.dram_tensor("qk", (P, KO, R), F32, kind="ExternalInput")
    pt = nc.dram_tensor("pt", (R, JS), F32, kind="ExternalInput")
    bm = nc.dram_tensor("bm", (P, KO, 2, JS), BF16, kind="ExternalInput")
    hs = nc.dram_tensor("hs", (B, JS), F32, kind="ExternalInput")
    ab = nc.dram_tensor("ab", (1, JS), F32, kind="ExternalInput")
    o = nc.dram_tensor("o", (B, JS), F32, kind="ExternalOutput")

    # b-tile sizes in k-chunks.  Tapered at both ends: small first tiles so
    # the PE can start as soon as possible, small last tiles so that after
    # the final DMA byte lands only one small tile's matmuls remain.
    TILES = tiles if tiles is not None else [1, 1, 2] + [4] * 14 + [2, 1, 1]
    assert sum(TILES) == KO
    MAXKT = max(TILES)

    with TileContext(nc) as tc:
        with (
            tc.tile_pool(name="persist", bufs=1) as persist,
            tc.tile_pool(name="bpool", bufs=bufs) as bpool,
            tc.tile_pool(name="psum", bufs=1, space="PSUM") as psum_pool,
        ):
            # Aux loads on the scalar HWDGE ring so the b stream on nc.sync
            # isn't delayed.  qk/ht-chunks/xh/xl are ordered so the PE's hq
            # matmul groups and first main tiles can start as early as
            # possible; hq groups are interleaved between the first main
            # tiles to fill the PE while the DMA stream warms up.
            xh_sb = persist.tile([P, KO, B], BF16)
            xl_sb = persist.tile([P, KO, B], BF16)
            qk_sb = persist.tile([P, KO, R], F32)
            ht_sb = persist.tile([P, KO, B], F32)
            pt_sb = persist.tile([R, JS], F32)
            hs_sb = persist.tile([B, JS], F32)
            a1_sb = persist.tile([1, JS], F32)
            ab_sb = persist.tile([B, JS], F32)

            def emit_aux():
                nc.scalar.dma_start(out=xh_sb[:], in_=xh[:, :, :])
                nc.scalar.dma_start(out=xl_sb[:], in_=xl[:, :, :])
                nc.scalar.dma_start(out=qk_sb[:], in_=qk[:, :, :])
                HT_CH = KO // 4
                for hc in range(4):
                    ksl = slice(hc * HT_CH, (hc + 1) * HT_CH)
                    nc.scalar.dma_start(out=ht_sb[:, ksl], in_=ht[:, ksl, :])
                nc.scalar.dma_start(out=pt_sb[:], in_=pt[:, :])
                nc.scalar.dma_start(out=hs_sb[:], in_=hs[:, :])
                # a_diag slice arrives as one row; broadcast to all 64 batch
                # partitions on the (otherwise idle) GPSIMD engine.
                nc.scalar.dma_start(out=a1_sb[:], in_=ab[:, :])
                nc.gpsimd.partition_broadcast(ab_sb[:], a1_sb[:])

            out_sb = persist.tile([B, JS], F32)
            hqt_sb = persist.tile([R, B], F32)

            import contextlib

            loop_ctx = (
                tc.For_i(0, loop_n, 1, hint_engines=(mybir.EngineType.PE,))
                if loop_n
                else contextlib.nullcontext()
            )
            if not (loop_n and aux_in_loop):
                emit_aux()
            with loop_ctx:
                if loop_n and aux_in_loop:
                    emit_aux()
                _emit_body(
                    nc, tc, TILES, MAXKT, bpool, psum_pool, persist,
                    qk_sb, ht_sb, xh_sb, xl_sb, pt_sb, hs_sb, ab_sb,
                    out_sb, hqt_sb, bm, o, hq_tiles, rank4_tile, psum_split4,
                )

    nc.finalize()
    return nc


def _emit_body(
    nc, tc, TILES, MAXKT, bpool, psum_pool, persist,
    qk_sb, ht_sb, xh_sb, xl_sb, pt_sb, hs_sb, ab_sb,
    out_sb, hqt_sb, bm, o, hq_tiles, rank4_tile, psum_split4=False,
):
            ps0 = psum_pool.tile([B, 512], F32)
            ps1 = psum_pool.tile([B, 512], F32)
            ps2 = psum_pool.tile([B, 512], F32)
            ps3 = psum_pool.tile([B, 512], F32)
            pshq = psum_pool.tile([R, B], F32)

            # Diagonal term early (off the critical tail).
            nc.vector.tensor_mul(out=out_sb[:], in0=hs_sb[:], in1=ab_sb[:])

            hq_done = [0]

            def hq_emit(n):
                # hqT = q^T @ h^T: emit the next n k-chunks (fp32).
                for ko in range(hq_done[0], min(hq_done[0] + n, KO)):
                    nc.tensor.matmul(
                        pshq[:],
                        qk_sb[:, ko],
                        ht_sb[:, ko],
                        start=(ko == 0),
                        stop=(ko == KO - 1),
                    )
                hq_done[0] = min(hq_done[0] + n, KO)

            def hq_group(g):
                hq_emit(16)

            # Main stream: x @ b_slice via 3-pass split-bf16.
            ko = 0
            for t, kt in enumerate(TILES):
                if hq_tiles[0] <= t < hq_tiles[1]:
                    ng = hq_tiles[1] - hq_tiles[0]
                    # Spread the 64 hq matmuls evenly over the window so
                    # they fill the PE's per-tile DMA-wait bubbles.
                    per = (KO + ng - 1) // ng
                    hq_emit(per)
                if t == rank4_tile:
                    hq_emit(KO)  # any remainder before the rank-4 term
                    # Rank-4 term into its own PSUM banks, mid-stream.
                    nc.vector.tensor_copy(out=hqt_sb[:], in_=pshq[:])
                    nc.tensor.matmul(
                        ps2[:], hqt_sb[:], pt_sb[:, 0:512], start=True, stop=True
                    )
                    nc.tensor.matmul(
                        ps3[:], hqt_sb[:], pt_sb[:, 512:JS], start=True, stop=True
                    )
                bfull = bpool.tile([P, MAXKT, 2, JS], BF16, name="btile")
                btile = bfull[:, :kt]
                dma_eng = nc.sync if t % 2 == 0 else nc.scalar
                dma_eng.dma_start(out=btile[:], in_=bm[:, ko : ko + kt])
                for k4 in range(kt):
                    st = ko == 0
                    lst = ko == KO - 1
                    bh = btile[:, k4, 0]
                    bl = btile[:, k4, 1]
                    if psum_split4:
                        # 4x N=256 matmuls per pass: marginally finer
                        # PE/DMA lockstep granularity (sim: -315 ns).
                        for qi, pq in enumerate((ps0, ps1)):
                            for hf in (0, 1):
                                sl = slice((2 * qi + hf) * 256, (2 * qi + hf + 1) * 256)
                                po = pq[:, hf * 256 : (hf + 1) * 256]
                                nc.tensor.matmul(
                                    po, xh_sb[:, ko], bh[:, sl], start=st, stop=False
                                )
                                nc.tensor.matmul(
                                    po, xh_sb[:, ko], bl[:, sl], start=False, stop=False
                                )
                                nc.tensor.matmul(
                                    po, xl_sb[:, ko], bh[:, sl], start=False, stop=lst
                                )
                        ko += 1
                        continue
                    nc.tensor.matmul(
                        ps0[:], xh_sb[:, ko], bh[:, 0:512], start=st, stop=False
                    )
                    nc.tensor.matmul(
                        ps1[:], xh_sb[:, ko], bh[:, 512:JS], start=st, stop=False
                    )
                    nc.tensor.matmul(
                        ps0[:], xh_sb[:, ko], bl[:, 0:512], start=False, stop=False
                    )
                    nc.tensor.matmul(
                        ps1[:], xh_sb[:, ko], bl[:, 512:JS], start=False, stop=False
                    )
                    # xl-stationary last: xl arrives after xh at startup.
                    nc.tensor.matmul(
                        ps0[:], xl_sb[:, ko], bh[:, 0:512], start=False, stop=lst
                    )
                    nc.tensor.matmul(
                        ps1[:], xl_sb[:, ko], bh[:, 512:JS], start=False, stop=lst
                    )
                    ko += 1

            # Rank-4 folded into out_sb mid-stream (off the critical tail).
            nc.vector.tensor_add(
                out=out_sb[:, 0:512], in0=out_sb[:, 0:512], in1=ps2[:]
            )
            nc.vector.tensor_add(
                out=out_sb[:, 512:JS], in0=out_sb[:, 512:JS], in1=ps3[:]
            )

            # Tail: fold the main accumulators and store.
            nc.vector.tensor_add(
                out=out_sb[:, 0:512], in0=out_sb[:, 0:512], in1=ps0[:]
            )
            nc.sync.dma_start(out=o[:, 0:512], in_=out_sb[:, 0:512])
            nc.vector.tensor_add(
                out=out_sb[:, 512:JS], in0=out_sb[:, 512:JS], in1=ps1[:]
            )
            nc.scalar.dma_start(out=o[:, 512:JS], in_=out_sb[:, 512:JS])


_NC_CACHE = None


def _get_nc() -> bass.Bass:
    global _NC_CACHE
    if _NC_CACHE is None:
        _NC_CACHE = _build_nc()
    return _NC_CACHE


def _split_bf16(a: np.ndarray) -> tuple[np.ndarray, np.ndarray]:
    hi = a.astype(BF)
    lo = (a - hi.astype(np.float32)).astype(BF)
    return hi, lo


def _in_maps(h, x, a_diag, p_vec, q_vec, b_mat):
    # Replicated inputs, pre-permuted to k-on-partitions chunk layout.
    # xt[ki, ko, b] = x[b, ko*128 + ki]
    xt = np.ascontiguousarray(x.reshape(B, KO, P).transpose(2, 1, 0))
    xh, xl = _split_bf16(xt)
    ht = np.ascontiguousarray(h.reshape(B, KO, P).transpose(2, 1, 0))
    # qk[ki, ko, r] = q_vec[ko*128 + ki, r]
    qk = np.ascontiguousarray(q_vec.reshape(KO, P, R).transpose(1, 0, 2))

    # b4[ko, ki, c, j] = b_mat[ko*128 + ki, c*1024 + j]
    b4 = b_mat.reshape(KO, P, NCORES, JS)

    in_maps = []
    for c in range(NCORES):
        j0 = c * JS
        bc = np.ascontiguousarray(b4[:, :, c, :].transpose(1, 0, 2))  # (P, KO, JS)
        bh, bl = _split_bf16(bc)
        bhl = np.ascontiguousarray(np.stack([bh, bl], axis=2))  # (P, KO, 2, JS)
        in_maps.append(
            {
                "xh": xh,
                "xl": xl,
                "ht": ht,
                "qk": qk,
                "pt": np.ascontiguousarray(p_vec[j0 : j0 + JS, :].T),
                "bm": bhl,
                "hs": np.ascontiguousarray(h[:, j0 : j0 + JS]),
                "ab": np.ascontiguousarray(a_diag[j0 : j0 + JS]).reshape(1, JS),
            }
        )
    return in_maps


def kernel(h, x, a_diag, p_vec, q_vec, b_mat) -> np.ndarray:
    h = np.ascontiguousarray(np.asarray(h, dtype=np.float32))
    x = np.ascontiguousarray(np.asarray(x, dtype=np.float32))
    a_diag = np.asarray(a_diag, dtype=np.float32)
    p_vec = np.asarray(p_vec, dtype=np.float32)
    q_vec = np.asarray(q_vec, dtype=np.float32)
    b_mat = np.asarray(b_mat, dtype=np.float32)

    nc = _get_nc()
    res = run_bass_kernel_spmd(
        nc, _in_maps(h, x, a_diag, p_vec, q_vec, b_mat), core_ids=list(range(NCORES))
    )
    return np.concatenate([r["o"] for r in res.results], axis=1)



# revision 8
# speedup vs baseline: 1.9403x; 1.9403x over previous
"""DPLR SSM block kernel for Trainium2, 8 NeuronCores.

Math:  out = h @ (diag(a_diag) + p q^T).T + x @ b_mat          (B=64, H=8192, R=4)
           = h * a_diag  +  (h @ q) @ p^T  +  x @ b_mat

The dense (H,H) DPLR matrix is never materialized.  The memory-bound part is
streaming b_mat.  Sharding: b_mat columns (= output features) are split 8
ways; each core computes out[:, c*1024:(c+1)*1024].  x/h/q are replicated;
the host pre-permutes everything into the k-on-partitions chunk layout the
tensor engine wants, so no on-device transposes are needed.

Correctness gate is rel_err < 2e-2, so b/x/h are carried as plain bf16
(measured end-to-end rel err ~2.4e-3, an 8x margin).  That makes the b
stream 2 bytes/element -> 16 MB/core, and the whole kernel a pure DMA-
roofline problem: ~18.4 MB/core at the ~360 GB/s per-core DMA ceiling is
~53 us of serialized transfer time (TRN2 DMA transfers serialize on the 16
shared DMA engines regardless of how many HWDGE queues issue them).

The tensor engine runs single-pass bf16 matmuls (~28 us total) and is far
off the critical path.  PSUM banks are pre-initialized by the DVE with the
diagonal term (h_slice * a_slice), the rank-4 term and the b matmuls then
accumulate onto them (start=False), so after the final b chunk lands the
tail is just: last 2 matmuls -> two PSUM->SBUF copies on DVE+Act in
parallel -> output DMA.

Per core c (j0 = c*1024):
  pshq (4, 64)      = sum_ko  q[ko]^T(128x4)^T . hT[ko](128x64)       [PE bf16]
  ps0/ps1 (64, 512) = hs * a  (DVE pre-init)
                    + hqT^T(64x4) . pT(4x512)                         [PE bf16]
                    + sum_ko x[ko]^T . b[ko]                          [PE bf16]
  out (64, 1024)    = copy(ps0) | copy(ps1)                           [DVE|Act]
"""

import ml_dtypes
import numpy as np

import concourse.bass as bass
import concourse.mybir as mybir
from concourse import bacc
from concourse.bass_utils import run_bass_kernel_spmd
from concourse.tile import TileContext

H = 8192
R = 4
B = 64
NCORES = 8
JS = H // NCORES  # 1024 output columns per core
P = 128
KO = H // P  # 64 k-chunks

F32 = mybir.dt.float32
BF16 = mybir.dt.bfloat16
BF = ml_dtypes.bfloat16


def _build_nc(
    tiles: list[int] | None = None,
    bufs: int = 6,
    hq_tiles: tuple[int, int] = (2, 7),
    rank4_tile: int = 8,
    num_devices: int = NCORES,
) -> bass.Bass:
    nc = bacc.Bacc("TRN2", target_bir_lowering=False, debug=False, num_devices=num_devices)

    xt = nc.dram_tensor("xt", (P, KO, B), BF16, kind="ExternalInput")
    ht = nc.dram_tensor("ht", (P, KO, B), BF16, kind="ExternalInput")
    qk = nc.dram_tensor("qk", (P, KO, R), BF16, kind="ExternalInput")
    pt = nc.dram_tensor("pt", (R, JS), BF16, kind="ExternalInput")
    bm = nc.dram_tensor("bm", (P, KO, JS), BF16, kind="ExternalInput")
    hs = nc.dram_tensor("hs", (B, JS), BF16, kind="ExternalInput")
    ab = nc.dram_tensor("ab", (1, JS), F32, kind="ExternalInput")
    o = nc.dram_tensor("o", (B, JS), F32, kind="ExternalOutput")

    # b-tile sizes in k-chunks, end-tapered so that after the final DMA byte
    # lands only one half-chunk's matmul remains before the tail copies.  The
    # final chunk (KO-1) is streamed as two half-width (512-col) DMAs so each
    # PSUM bank's last matmul + copy chain starts as early as possible.
    TILES = tiles if tiles is not None else [4] * 14 + [2, 2, 2, 1]
    assert sum(TILES) == KO - 1
    MAXKT = max(TILES)

    with TileContext(nc) as tc:
        with (
            tc.tile_pool(name="persist", bufs=1) as persist,
            tc.tile_pool(name="bpool", bufs=bufs) as bpool,
            tc.tile_pool(name="psum", bufs=1, space="PSUM") as psum_pool,
        ):
            xt_sb = persist.tile([P, KO, B], BF16)
            ht_sb = persist.tile([P, KO, B], BF16)
            qk_sb = persist.tile([P, KO, R], BF16)
            pt_sb = persist.tile([R, JS], BF16)
            hs_sb = persist.tile([B, JS], BF16)
            a1_sb = persist.tile([1, JS], F32)
            ab_sb = persist.tile([B, JS], F32)
            hqt_sb = persist.tile([R, B], BF16)
            o_sb = persist.tile([B, JS], F32)

            ps0 = psum_pool.tile([B, 512], F32)
            ps1 = psum_pool.tile([B, 512], F32)
            pshq = psum_pool.tile([R, B], F32)

            # Aux loads all go on the Activation queue; the SP queue leads
            # with the first b tile so the shared DMA engines never idle at
            # the head.  All transfers serialize on the shared DMA engines so
            # ordering does not change the total stream time; PE has ~2x
            # slack and catches up from any startup stall.
            nc.scalar.dma_start(out=xt_sb[:], in_=xt[:, :, :])
            nc.scalar.dma_start(out=hs_sb[:], in_=hs[:, :])
            nc.scalar.dma_start(out=a1_sb[:], in_=ab[:, :])
            nc.scalar.dma_start(out=qk_sb[:], in_=qk[:, :, :])
            nc.scalar.dma_start(out=ht_sb[:], in_=ht[:, :, :])
            nc.scalar.dma_start(out=pt_sb[:], in_=pt[:, :])

            # a_diag slice arrives as one row; broadcast to all 64 batch
            # partitions on the (otherwise idle) GPSIMD engine, then write
            # the diagonal term into PSUM as the accumulation seed.
            nc.gpsimd.partition_broadcast(ab_sb[:], a1_sb[:])
            nc.vector.tensor_mul(out=ps0[:], in0=hs_sb[:, 0:512], in1=ab_sb[:, 0:512])
            nc.vector.tensor_mul(out=ps1[:], in0=hs_sb[:, 512:JS], in1=ab_sb[:, 512:JS])

            hq_done = [0]

            def hq_emit(n):
                # hqT = q^T @ h^T: emit the next n k-chunks (bf16).
                for k in range(hq_done[0], min(hq_done[0] + n, KO)):
                    nc.tensor.matmul(
                        pshq[:],
                        qk_sb[:, k],
                        ht_sb[:, k],
                        start=(k == 0),
                        stop=(k == KO - 1),
                    )
                hq_done[0] = min(hq_done[0] + n, KO)

            # Main stream: x @ b_slice, single-pass bf16, accumulating onto
            # the pre-seeded PSUM banks.
            ko = 0
            for t, kt in enumerate(TILES):
                bfull = bpool.tile([P, MAXKT, JS], BF16, name="btile")
                btile = bfull[:, :kt]
                dma_eng = nc.sync if t % 2 == 0 else nc.scalar
                dma_eng.dma_start(out=btile[:], in_=bm[:, ko : ko + kt])
                for k4 in range(kt):
                    nc.tensor.matmul(
                        ps0[:], xt_sb[:, ko], btile[:, k4, 0:512], start=False, stop=False
                    )
                    nc.tensor.matmul(
                        ps1[:], xt_sb[:, ko], btile[:, k4, 512:JS], start=False, stop=False
                    )
                    ko += 1
                if hq_tiles[0] <= t < hq_tiles[1]:
                    # Spread the 64 hq matmuls over the window so they fill
                    # the PE's per-tile DMA-wait bubbles.
                    ng = hq_tiles[1] - hq_tiles[0]
                    hq_emit((KO + ng - 1) // ng)
                if t == rank4_tile:
                    hq_emit(KO)  # any remainder before the rank-4 term
                    # Rank-4 term accumulated straight into the main banks.
                    nc.vector.tensor_copy(out=hqt_sb[:], in_=pshq[:])
                    nc.tensor.matmul(
                        ps0[:], hqt_sb[:], pt_sb[:, 0:512], start=False, stop=False
                    )
                    nc.tensor.matmul(
                        ps1[:], hqt_sb[:], pt_sb[:, 512:JS], start=False, stop=False
                    )

            # Final chunk, streamed per half so each bank's copy starts early.
            assert ko == KO - 1
            bl_a = bpool.tile([P, MAXKT, JS], BF16, name="btile")
            nc.sync.dma_start(out=bl_a[:, 0, 0:512], in_=bm[:, KO - 1, 0:512])
            nc.tensor.matmul(
                ps0[:], xt_sb[:, KO - 1], bl_a[:, 0, 0:512], start=False, stop=True
            )
            nc.vector.tensor_copy(out=o_sb[:, 0:512], in_=ps0[:])
            nc.scalar.dma_start(out=bl_a[:, 0, 512:JS], in_=bm[:, KO - 1, 512:JS])
            nc.tensor.matmul(
                ps1[:], xt_sb[:, KO - 1], bl_a[:, 0, 512:JS], start=False, stop=True
            )
            nc.scalar.copy(out=o_sb[:, 512:JS], in_=ps1[:])
            # Single store: one HWDGE issue chain instead of two.
            nc.sync.dma_start(out=o[:, :], in_=o_sb[:, :])

    nc.finalize()
    return nc


_NC_CACHE = None


def _get_nc() -> bass.Bass:
    global _NC_CACHE
    if _NC_CACHE is None:
        _NC_CACHE = _build_nc()
    return _NC_CACHE


def _in_maps(h, x, a_diag, p_vec, q_vec, b_mat):
    # Replicated inputs, pre-permuted to k-on-partitions chunk layout.
    # xt[ki, ko, b] = x[b, ko*128 + ki]
    xt = np.ascontiguousarray(x.reshape(B, KO, P).transpose(2, 1, 0)).astype(BF)
    ht = np.ascontiguousarray(h.reshape(B, KO, P).transpose(2, 1, 0)).astype(BF)
    # qk[ki, ko, r] = q_vec[ko*128 + ki, r]
    qk = np.ascontiguousarray(q_vec.reshape(KO, P, R).transpose(1, 0, 2)).astype(BF)

    # b4[ko, ki, c, j] = b_mat[ko*128 + ki, c*1024 + j]
    b4 = b_mat.astype(BF).reshape(KO, P, NCORES, JS)

    in_maps = []
    for c in range(NCORES):
        j0 = c * JS
        bc = np.ascontiguousarray(b4[:, :, c, :].transpose(1, 0, 2))  # (P, KO, JS)
        in_maps.append(
            {
                "xt": xt,
                "ht": ht,
                "qk": qk,
                "pt": np.ascontiguousarray(p_vec[j0 : j0 + JS, :].T).astype(BF),
                "bm": bc,
                "hs": np.ascontiguousarray(h[:, j0 : j0 + JS]).astype(BF),
                "ab": np.ascontiguousarray(a_diag[j0 : j0 + JS]).reshape(1, JS),
            }
        )
    return in_maps


def kernel(h, x, a_diag, p_vec, q_vec, b_mat) -> np.ndarray:
    h = np.ascontiguousarray(np.asarray(h, dtype=np.float32))
    x = np.ascontiguousarray(np.asarray(x, dtype=np.float32))
    a_diag = np.asarray(a_diag, dtype=np.float32)
    p_vec = np.asarray(p_vec, dtype=np.float32)
    q_vec = np.asarray(q_vec, dtype=np.float32)
    b_mat = np.asarray(b_mat, dtype=np.float32)

    nc = _get_nc()
    res = run_bass_kernel_spmd(
        nc, _in_maps(h, x, a_diag, p_vec, q_vec, b_mat), core_ids=list(range(NCORES))
    )
    return np.concatenate([r["o"] for r in res.results], axis=1)


# revision 11
# speedup vs baseline: 1.9880x; 1.0246x over previous
"""DPLR SSM block kernel for Trainium2, 8 NeuronCores.

Math:  out = h @ (diag(a_diag) + p q^T).T + x @ b_mat          (B=64, H=8192, R=4)
           = h * a_diag  +  (h @ q) @ p^T  +  x @ b_mat

The dense (H,H) DPLR matrix is never materialized.  The memory-bound part is
streaming b_mat.  Sharding: b_mat columns (= output features) are split 8
ways; each core computes out[:, c*1024:(c+1)*1024].  x/h/q are replicated;
the host pre-permutes everything into the k-on-partitions chunk layout the
tensor engine wants, so no on-device transposes are needed.

Correctness gate is rel_err < 2e-2, so b/x/h are carried as plain bf16
(measured end-to-end rel err ~2.4e-3, an 8x margin).  That makes the b
stream 2 bytes/element -> 16 MB/core, and the whole kernel a pure DMA-
roofline problem: ~18.4 MB/core at the ~360 GB/s per-core DMA ceiling is
~53 us of serialized transfer time (TRN2 DMA transfers serialize on the 16
shared DMA engines regardless of how many HWDGE queues issue them).

The tensor engine runs single-pass bf16 matmuls (~28 us total) and is far
off the critical path.  PSUM banks are pre-initialized by the DVE with the
diagonal term (h_slice * a_slice), the rank-4 term and the b matmuls then
accumulate onto them (start=False), so after the final b chunk lands the
tail is just: last 2 matmuls -> two PSUM->SBUF copies on DVE+Act in
parallel -> output DMA.

Per core c (j0 = c*1024):
  pshq (4, 64)      = sum_ko  q[ko]^T(128x4)^T . hT[ko](128x64)       [PE bf16]
  ps0/ps1 (64, 512) = hs * a  (DVE pre-init)
                    + hqT^T(64x4) . pT(4x512)                         [PE bf16]
                    + sum_ko x[ko]^T . b[ko]                          [PE bf16]
  out (64, 1024)    = copy(ps0) | copy(ps1)                           [DVE|Act]
"""

import ml_dtypes
import numpy as np

import concourse.bass as bass
import concourse.mybir as mybir
from concourse import bacc
from concourse.bass_utils import run_bass_kernel_spmd
from concourse.tile import TileContext

H = 8192
R = 4
B = 64
NCORES = 8
JS = H // NCORES  # 1024 output columns per core
P = 128
KO = H // P  # 64 k-chunks

F32 = mybir.dt.float32
BF16 = mybir.dt.bfloat16
FP8 = mybir.dt.float8e4
BF = ml_dtypes.bfloat16
F8 = ml_dtypes.float8_e4m3


def _build_nc(
    tiles: list[int] | None = None,
    bufs: int = 6,
    hq_tiles: tuple[int, int] = (2, 7),
    rank4_tile: int = 8,
    num_devices: int = NCORES,
) -> bass.Bass:
    nc = bacc.Bacc("TRN2", target_bir_lowering=False, debug=False, num_devices=num_devices)

    xt = nc.dram_tensor("xt", (P, KO, B), BF16, kind="ExternalInput")
    ht = nc.dram_tensor("ht", (P, KO, B), FP8, kind="ExternalInput")
    qk = nc.dram_tensor("qk", (P, KO, R), FP8, kind="ExternalInput")
    pt = nc.dram_tensor("pt", (R, JS), BF16, kind="ExternalInput")
    bm = nc.dram_tensor("bm", (P, KO, JS), BF16, kind="ExternalInput")
    hs = nc.dram_tensor("hs", (B, JS), BF16, kind="ExternalInput")
    ab = nc.dram_tensor("ab", (1, JS), F32, kind="ExternalInput")
    o = nc.dram_tensor("o", (B, JS), F32, kind="ExternalOutput")

    # b-tile sizes in k-chunks, end-tapered so that after the final DMA byte
    # lands only one half-chunk's matmul remains before the tail copies.  The
    # final chunk (KO-1) is streamed as two half-width (512-col) DMAs so each
    # PSUM bank's last matmul + copy chain starts as early as possible.
    TILES = tiles if tiles is not None else [4] * 14 + [2, 2, 2, 1]
    assert sum(TILES) == KO - 1
    MAXKT = max(TILES)

    with TileContext(nc) as tc:
        with (
            tc.tile_pool(name="persist", bufs=1) as persist,
            tc.tile_pool(name="bpool", bufs=bufs) as bpool,
            tc.tile_pool(name="psum", bufs=1, space="PSUM") as psum_pool,
        ):
            xt_sb = persist.tile([P, KO, B], BF16)
            ht_sb = persist.tile([P, KO, B], FP8)
            qk_sb = persist.tile([P, KO, R], FP8)
            pt_sb = persist.tile([R, JS], BF16)
            hs_sb = persist.tile([B, JS], BF16)
            a1_sb = persist.tile([1, JS], F32)
            ab_sb = persist.tile([B, JS], F32)
            hqt_sb = persist.tile([R, B], BF16)
            o_sb = persist.tile([B, JS], F32)

            ps0 = psum_pool.tile([B, 512], F32)
            ps1 = psum_pool.tile([B, 512], F32)
            pshq = psum_pool.tile([R, B], F32)

            # Aux loads all go on the Activation queue; the SP queue leads
            # with the first b tile so the shared DMA engines never idle at
            # the head.  All transfers serialize on the shared DMA engines so
            # ordering does not change the total stream time; PE has ~2x
            # slack and catches up from any startup stall.
            nc.scalar.dma_start(out=xt_sb[:], in_=xt[:, :, :])
            nc.scalar.dma_start(out=hs_sb[:], in_=hs[:, :])
            nc.scalar.dma_start(out=a1_sb[:], in_=ab[:, :])
            nc.scalar.dma_start(out=qk_sb[:], in_=qk[:, :, :])
            nc.scalar.dma_start(out=ht_sb[:], in_=ht[:, :, :])
            nc.scalar.dma_start(out=pt_sb[:], in_=pt[:, :])

            # a_diag slice arrives as one row; broadcast to all 64 batch
            # partitions on the (otherwise idle) GPSIMD engine, then write
            # the diagonal term into PSUM as the accumulation seed.
            nc.gpsimd.partition_broadcast(ab_sb[:], a1_sb[:])
            nc.vector.tensor_mul(out=ps0[:], in0=hs_sb[:, 0:512], in1=ab_sb[:, 0:512])
            nc.vector.tensor_mul(out=ps1[:], in0=hs_sb[:, 512:JS], in1=ab_sb[:, 512:JS])

            hq_done = [0]

            def hq_emit(n):
                # hqT = q^T @ h^T: emit the next n k-chunks (bf16).
                for k in range(hq_done[0], min(hq_done[0] + n, KO)):
                    nc.tensor.matmul(
                        pshq[:],
                        qk_sb[:, k],
                        ht_sb[:, k],
                        start=(k == 0),
                        stop=(k == KO - 1),
                    )
                hq_done[0] = min(hq_done[0] + n, KO)

            # Main stream: x @ b_slice, single-pass bf16, accumulating onto
            # the pre-seeded PSUM banks.
            ko = 0
            for t, kt in enumerate(TILES):
                bfull = bpool.tile([P, MAXKT, JS], BF16, name="btile")
                btile = bfull[:, :kt]
                dma_eng = nc.sync if t % 2 == 0 else nc.scalar
                dma_eng.dma_start(out=btile[:], in_=bm[:, ko : ko + kt])
                for k4 in range(kt):
                    nc.tensor.matmul(
                        ps0[:], xt_sb[:, ko], btile[:, k4, 0:512], start=False, stop=False
                    )
                    nc.tensor.matmul(
                        ps1[:], xt_sb[:, ko], btile[:, k4, 512:JS], start=False, stop=False
                    )
                    ko += 1
                if hq_tiles[0] <= t < hq_tiles[1]:
                    # Spread the 64 hq matmuls over the window so they fill
                    # the PE's per-tile DMA-wait bubbles.
                    ng = hq_tiles[1] - hq_tiles[0]
                    hq_emit((KO + ng - 1) // ng)
                if t == rank4_tile:
                    hq_emit(KO)  # any remainder before the rank-4 term
                    # Rank-4 term accumulated straight into the main banks.
                    nc.vector.tensor_copy(out=hqt_sb[:], in_=pshq[:])
                    nc.tensor.matmul(
                        ps0[:], hqt_sb[:], pt_sb[:, 0:512], start=False, stop=False
                    )
                    nc.tensor.matmul(
                        ps1[:], hqt_sb[:], pt_sb[:, 512:JS], start=False, stop=False
                    )

            # Final chunk, streamed per half so each bank's copy starts early.
            assert ko == KO - 1
            bl_a = bpool.tile([P, MAXKT, JS], BF16, name="btile")
            nc.sync.dma_start(out=bl_a[:, 0, 0:512], in_=bm[:, KO - 1, 0:512])
            nc.tensor.matmul(
                ps0[:], xt_sb[:, KO - 1], bl_a[:, 0, 0:512], start=False, stop=True
            )
            nc.vector.tensor_copy(out=o_sb[:, 0:512], in_=ps0[:])
            nc.scalar.dma_start(out=bl_a[:, 0, 512:JS], in_=bm[:, KO - 1, 512:JS])
            nc.tensor.matmul(
                ps1[:], xt_sb[:, KO - 1], bl_a[:, 0, 512:JS], start=False, stop=True
            )
            nc.scalar.copy(out=o_sb[:, 512:JS], in_=ps1[:])
            # Single store: one HWDGE issue chain instead of two.
            nc.sync.dma_start(out=o[:, :], in_=o_sb[:, :])

    nc.finalize()
    return nc


_NC_CACHE = None


def _get_nc() -> bass.Bass:
    global _NC_CACHE
    if _NC_CACHE is None:
        _NC_CACHE = _build_nc()
    return _NC_CACHE


def _in_maps(h, x, a_diag, p_vec, q_vec, b_mat):
    # Replicated inputs, pre-permuted to k-on-partitions chunk layout.
    # xt[ki, ko, b] = x[b, ko*128 + ki]
    xt = np.ascontiguousarray(x.reshape(B, KO, P).transpose(2, 1, 0)).astype(BF)
    # h/q are only used for the tiny rank-4 term (~2.6%% of output
    # magnitude), so fp8 e4m3 is plenty.  q values (~1.6e-2) sit in
    # e4m3 subnormal range, so scale q by 2^6 and p by 2^-6 (exact).
    ht = np.ascontiguousarray(h.reshape(B, KO, P).transpose(2, 1, 0)).astype(F8)
    # qk[ki, ko, r] = q_vec[ko*128 + ki, r]
    qk = np.ascontiguousarray(q_vec.reshape(KO, P, R).transpose(1, 0, 2) * 64.0).astype(F8)

    # b4[ko, ki, c, j] = b_mat[ko*128 + ki, c*1024 + j]
    b4 = b_mat.astype(BF).reshape(KO, P, NCORES, JS)

    in_maps = []
    for c in range(NCORES):
        j0 = c * JS
        bc = np.ascontiguousarray(b4[:, :, c, :].transpose(1, 0, 2))  # (P, KO, JS)
        in_maps.append(
            {
                "xt": xt,
                "ht": ht,
                "qk": qk,
                "pt": np.ascontiguousarray(p_vec[j0 : j0 + JS, :].T / 64.0).astype(BF),
                "bm": bc,
                "hs": np.ascontiguousarray(h[:, j0 : j0 + JS]).astype(BF),
                "ab": np.ascontiguousarray(a_diag[j0 : j0 + JS]).reshape(1, JS),
            }
        )
    return in_maps


def kernel(h, x, a_diag, p_vec, q_vec, b_mat) -> np.ndarray:
    h = np.ascontiguousarray(np.asarray(h, dtype=np.float32))
    x = np.ascontiguousarray(np.asarray(x, dtype=np.float32))
    a_diag = np.asarray(a_diag, dtype=np.float32)
    p_vec = np.asarray(p_vec, dtype=np.float32)
    q_vec = np.asarray(q_vec, dtype=np.float32)
    b_mat = np.asarray(b_mat, dtype=np.float32)

    nc = _get_nc()
    res = run_bass_kernel_spmd(
        nc, _in_maps(h, x, a_diag, p_vec, q_vec, b_mat), core_ids=list(range(NCORES))
    )
    return np.concatenate([r["o"] for r in res.results], axis=1)


# revision 15
# speedup vs baseline: 1.9970x; 1.0045x over previous
"""DPLR SSM block kernel for Trainium2, 8 NeuronCores.

Math:  out = h @ (diag(a_diag) + p q^T).T + x @ b_mat          (B=64, H=8192, R=4)
           = h * a_diag  +  (h @ q) @ p^T  +  x @ b_mat

The dense (H,H) DPLR matrix is never materialized.  The memory-bound part is
streaming b_mat.  Sharding: b_mat columns (= output features) are split 8
ways; each core computes out[:, c*1024:(c+1)*1024].  x/h/q are replicated;
the host pre-permutes everything into the k-on-partitions chunk layout the
tensor engine wants, so no on-device transposes are needed.

Correctness gate is rel_err < 2e-2, so b/x are carried as plain bf16 and
h/q (which only feed the rank-4 and diagonal terms, ~3% of the output
magnitude) as fp8 e4m3; measured end-to-end rel err ~2.6e-3, a 7.7x
margin.  That makes the b stream 2 bytes/element -> 16 MB/core, and the
whole kernel a pure DMA-roofline problem: ~18.3 MB/core at the ~360 GB/s
per-core DMA ceiling is ~52 us of serialized transfer time (TRN2 DMA
transfers serialize on the 16 shared DMA engines regardless of how many
HWDGE queues issue them; measured zero inter-transfer gaps).

The tensor engine runs single-pass bf16 matmuls (~28 us total) and is far
off the critical path.  PSUM banks are pre-initialized by the DVE with the
diagonal term (h_slice * a_slice), the rank-4 term and the b matmuls then
accumulate onto them (start=False), so after the final b half-chunk lands
the tail is just: sem(900) -> last matmul -> PSUM->SBUF copies on DVE+Act
in parallel -> one output DMA (issue ~1.5us) -> sem + epilogue barriers.
TimelineSim: 59002 ns (baseline this replaced: 117827 ns).

Per core c (j0 = c*1024):
  pshq (4, 64)      = sum_ko  q[ko](128x4)^T . hT[ko](128x64)         [PE fp8]
  ps0/ps1 (64, 512) = hs * a  (DVE pre-init, fp8 x f32)
                    + hqT^T(64x4) . pT(4x512)                         [PE bf16]
                    + sum_ko x[ko]^T . b[ko]                          [PE bf16]
  out (64, 1024)    = copy(ps0) | copy(ps1)                           [DVE|Act]
"""

import ml_dtypes
import numpy as np

import concourse.bass as bass
import concourse.mybir as mybir
from concourse import bacc
from concourse.bass_utils import run_bass_kernel_spmd
from concourse.tile import TileContext

H = 8192
R = 4
B = 64
NCORES = 8
JS = H // NCORES  # 1024 output columns per core
P = 128
KO = H // P  # 64 k-chunks

F32 = mybir.dt.float32
BF16 = mybir.dt.bfloat16
FP8 = mybir.dt.float8e4
BF = ml_dtypes.bfloat16
F8 = ml_dtypes.float8_e4m3


def _build_nc(
    tiles: list[int] | None = None,
    bufs: int = 6,
    hq_tiles: tuple[int, int] = (2, 7),
    rank4_tile: int = 8,
    num_devices: int = NCORES,
) -> bass.Bass:
    nc = bacc.Bacc("TRN2", target_bir_lowering=False, debug=False, num_devices=num_devices)

    xt = nc.dram_tensor("xt", (P, KO, B), BF16, kind="ExternalInput")
    ht = nc.dram_tensor("ht", (P, KO, B), FP8, kind="ExternalInput")
    qk = nc.dram_tensor("qk", (P, KO, R), FP8, kind="ExternalInput")
    pt = nc.dram_tensor("pt", (R, JS), BF16, kind="ExternalInput")
    bm = nc.dram_tensor("bm", (P, KO, JS), BF16, kind="ExternalInput")
    hs = nc.dram_tensor("hs", (B, JS), FP8, kind="ExternalInput")
    ab = nc.dram_tensor("ab", (1, JS), F32, kind="ExternalInput")
    o = nc.dram_tensor("o", (B, JS), F32, kind="ExternalOutput")

    # b-tile sizes in k-chunks, end-tapered so that after the final DMA byte
    # lands only one half-chunk's matmul remains before the tail copies.  The
    # final chunk (KO-1) is streamed as two half-width (512-col) DMAs so each
    # PSUM bank's last matmul + copy chain starts as early as possible.
    TILES = tiles if tiles is not None else [4] * 14 + [2, 2, 1, 1, 1]
    assert sum(TILES) == KO - 1
    MAXKT = max(TILES)

    with TileContext(nc) as tc:
        with (
            tc.tile_pool(name="persist", bufs=1) as persist,
            tc.tile_pool(name="bpool", bufs=bufs) as bpool,
            tc.tile_pool(name="psum", bufs=1, space="PSUM") as psum_pool,
        ):
            xt_sb = persist.tile([P, KO, B], BF16)
            ht_sb = persist.tile([P, KO, B], FP8)
            qk_sb = persist.tile([P, KO, R], FP8)
            pt_sb = persist.tile([R, JS], BF16)
            hs_sb = persist.tile([B, JS], FP8)
            a1_sb = persist.tile([1, JS], F32)
            ab_sb = persist.tile([B, JS], F32)
            hqt_sb = persist.tile([R, B], BF16)
            o_sb = persist.tile([B, JS], F32)

            ps0 = psum_pool.tile([B, 512], F32)
            ps1 = psum_pool.tile([B, 512], F32)
            pshq = psum_pool.tile([R, B], F32)

            # Aux loads all go on the Activation queue; the SP queue leads
            # with the first b tile so the shared DMA engines never idle at
            # the head.  All transfers serialize on the shared DMA engines so
            # ordering does not change the total stream time; PE has ~2x
            # slack and catches up from any startup stall.
            nc.scalar.dma_start(out=xt_sb[:], in_=xt[:, :, :])
            nc.scalar.dma_start(out=hs_sb[:], in_=hs[:, :])
            nc.scalar.dma_start(out=a1_sb[:], in_=ab[:, :])
            nc.scalar.dma_start(out=qk_sb[:], in_=qk[:, :, :])
            nc.scalar.dma_start(out=ht_sb[:], in_=ht[:, :, :])
            nc.scalar.dma_start(out=pt_sb[:], in_=pt[:, :])

            # a_diag slice arrives as one row; broadcast to all 64 batch
            # partitions on the (otherwise idle) GPSIMD engine, then write
            # the diagonal term into PSUM as the accumulation seed.
            nc.gpsimd.partition_broadcast(ab_sb[:], a1_sb[:])
            nc.vector.tensor_mul(out=ps0[:], in0=hs_sb[:, 0:512], in1=ab_sb[:, 0:512])
            nc.vector.tensor_mul(out=ps1[:], in0=hs_sb[:, 512:JS], in1=ab_sb[:, 512:JS])

            hq_done = [0]

            def hq_emit(n):
                # hqT = q^T @ h^T: emit the next n k-chunks (fp8).
                for k in range(hq_done[0], min(hq_done[0] + n, KO)):
                    nc.tensor.matmul(
                        pshq[:],
                        qk_sb[:, k],
                        ht_sb[:, k],
                        start=(k == 0),
                        stop=(k == KO - 1),
                    )
                hq_done[0] = min(hq_done[0] + n, KO)

            # Main stream: x @ b_slice, single-pass bf16, accumulating onto
            # the pre-seeded PSUM banks.
            ko = 0
            for t, kt in enumerate(TILES):
                bfull = bpool.tile([P, MAXKT, JS], BF16, name="btile")
                btile = bfull[:, :kt]
                dma_eng = nc.sync if t % 2 == 0 else nc.scalar
                dma_eng.dma_start(out=btile[:], in_=bm[:, ko : ko + kt])
                for k4 in range(kt):
                    nc.tensor.matmul(
                        ps0[:], xt_sb[:, ko], btile[:, k4, 0:512], start=False, stop=False
                    )
                    nc.tensor.matmul(
                        ps1[:], xt_sb[:, ko], btile[:, k4, 512:JS], start=False, stop=False
                    )
                    ko += 1
                if hq_tiles[0] <= t < hq_tiles[1]:
                    # Spread the 64 hq matmuls over the window so they fill
                    # the PE's per-tile DMA-wait bubbles.
                    ng = hq_tiles[1] - hq_tiles[0]
                    hq_emit((KO + ng - 1) // ng)
                if t == rank4_tile:
                    hq_emit(KO)  # any remainder before the rank-4 term
                    # Rank-4 term accumulated straight into the main banks.
                    nc.vector.tensor_copy(out=hqt_sb[:], in_=pshq[:])
                    nc.tensor.matmul(
                        ps0[:], hqt_sb[:], pt_sb[:, 0:512], start=False, stop=False
                    )
                    nc.tensor.matmul(
                        ps1[:], hqt_sb[:], pt_sb[:, 512:JS], start=False, stop=False
                    )

            # Final chunk, streamed per half so each bank's copy starts early.
            assert ko == KO - 1
            bl_a = bpool.tile([P, MAXKT, JS], BF16, name="btile")
            nc.sync.dma_start(out=bl_a[:, 0, 0:512], in_=bm[:, KO - 1, 0:512])
            nc.tensor.matmul(
                ps0[:], xt_sb[:, KO - 1], bl_a[:, 0, 0:512], start=False, stop=True
            )
            nc.vector.tensor_copy(out=o_sb[:, 0:512], in_=ps0[:])
            nc.scalar.dma_start(out=bl_a[:, 0, 512:JS], in_=bm[:, KO - 1, 512:JS])
            nc.tensor.matmul(
                ps1[:], xt_sb[:, KO - 1], bl_a[:, 0, 512:JS], start=False, stop=True
            )
            nc.scalar.copy(out=o_sb[:, 512:JS], in_=ps1[:])
            # Single store: one HWDGE issue chain instead of two.
            nc.sync.dma_start(out=o[:, :], in_=o_sb[:, :])

    nc.finalize()
    return nc


_NC_CACHE = None


def _get_nc() -> bass.Bass:
    global _NC_CACHE
    if _NC_CACHE is None:
        _NC_CACHE = _build_nc()
    return _NC_CACHE


def _in_maps(h, x, a_diag, p_vec, q_vec, b_mat):
    # Replicated inputs, pre-permuted to k-on-partitions chunk layout.
    # xt[ki, ko, b] = x[b, ko*128 + ki]
    xt = np.ascontiguousarray(x.reshape(B, KO, P).transpose(2, 1, 0)).astype(BF)
    # h/q are only used for the tiny rank-4 term (~2.6% of output
    # magnitude), so fp8 e4m3 is plenty.  q values (~1.6e-2) sit in
    # e4m3 subnormal range, so scale q by 2^6 and p by 2^-6 (exact).
    ht = np.ascontiguousarray(h.reshape(B, KO, P).transpose(2, 1, 0)).astype(F8)
    # qk[ki, ko, r] = q_vec[ko*128 + ki, r]
    qk = np.ascontiguousarray(q_vec.reshape(KO, P, R).transpose(1, 0, 2) * 64.0).astype(F8)

    # b4[ko, ki, c, j] = b_mat[ko*128 + ki, c*1024 + j]
    b4 = b_mat.astype(BF).reshape(KO, P, NCORES, JS)

    in_maps = []
    for c in range(NCORES):
        j0 = c * JS
        bc = np.ascontiguousarray(b4[:, :, c, :].transpose(1, 0, 2))  # (P, KO, JS)
        in_maps.append(
            {
                "xt": xt,
                "ht": ht,
                "qk": qk,
                "pt": np.ascontiguousarray(p_vec[j0 : j0 + JS, :].T / 64.0).astype(BF),
                "bm": bc,
                "hs": np.ascontiguousarray(h[:, j0 : j0 + JS]).astype(F8),
                "ab": np.ascontiguousarray(a_diag[j0 : j0 + JS]).reshape(1, JS),
            }
        )
    return in_maps


def kernel(h, x, a_diag, p_vec, q_vec, b_mat) -> np.ndarray:
    h = np.ascontiguousarray(np.asarray(h, dtype=np.float32))
    x = np.ascontiguousarray(np.asarray(x, dtype=np.float32))
    a_diag = np.asarray(a_diag, dtype=np.float32)
    p_vec = np.asarray(p_vec, dtype=np.float32)
    q_vec = np.asarray(q_vec, dtype=np.float32)
    b_mat = np.asarray(b_mat, dtype=np.float32)

    nc = _get_nc()
    res = run_bass_kernel_spmd(
        nc, _in_maps(h, x, a_diag, p_vec, q_vec, b_mat), core_ids=list(range(NCORES))
    )
    return np.concatenate([r["o"] for r in res.results], axis=1)


# revision 16
# speedup vs baseline: 1.9994x; 1.0012x over previous
"""DPLR SSM block kernel for Trainium2, 8 NeuronCores.

Math:  out = h @ (diag(a_diag) + p q^T).T + x @ b_mat          (B=64, H=8192, R=4)
           = h * a_diag  +  (h @ q) @ p^T  +  x @ b_mat

The dense (H,H) DPLR matrix is never materialized.  The memory-bound part is
streaming b_mat.  Sharding: b_mat columns (= output features) are split 8
ways; each core computes out[:, c*1024:(c+1)*1024].  x/h/q are replicated;
the host pre-permutes everything into the k-on-partitions chunk layout the
tensor engine wants, so no on-device transposes are needed.

Correctness gate is rel_err < 2e-2, so b/x are carried as plain bf16 and
h/q (which only feed the rank-4 and diagonal terms, ~3% of the output
magnitude) as fp8 e4m3; measured end-to-end rel err ~2.6e-3, a 7.7x
margin.  That makes the b stream 2 bytes/element -> 16 MB/core, and the
whole kernel a pure DMA-roofline problem: ~18.3 MB/core at the ~360 GB/s
per-core DMA ceiling is ~52 us of serialized transfer time (TRN2 DMA
transfers serialize on the 16 shared DMA engines regardless of how many
HWDGE queues issue them; measured zero inter-transfer gaps).

The tensor engine runs single-pass bf16 matmuls (~28 us total) and is far
off the critical path.  PSUM banks are pre-initialized by the DVE with the
diagonal term (h_slice * a_slice), the rank-4 term and the b matmuls then
accumulate onto them (start=False), so after the final b half-chunk lands
the tail is just: sem(900) -> last matmul -> PSUM->SBUF copies on DVE+Act
in parallel -> one output DMA (issue ~1.5us) -> sem + epilogue barriers.
TimelineSim: 59002 ns (baseline this replaced: 117827 ns).

Per core c (j0 = c*1024):
  pshq (4, 64)      = sum_ko  q[ko](128x4)^T . hT[ko](128x64)         [PE fp8]
  ps0/ps1 (64, 512) = hs * a  (DVE pre-init, fp8 x f32)
                    + hqT^T(64x4) . pT(4x512)                         [PE bf16]
                    + sum_ko x[ko]^T . b[ko]                          [PE bf16]
  out (64, 1024)    = copy(ps0) | copy(ps1)                           [DVE|Act]
"""

import ml_dtypes
import numpy as np

import concourse.bass as bass
import concourse.mybir as mybir
from concourse import bacc
from concourse.bass_utils import run_bass_kernel_spmd
from concourse.tile import TileContext

H = 8192
R = 4
B = 64
NCORES = 8
JS = H // NCORES  # 1024 output columns per core
P = 128
KO = H // P  # 64 k-chunks

F32 = mybir.dt.float32
BF16 = mybir.dt.bfloat16
FP8 = mybir.dt.float8e4
BF = ml_dtypes.bfloat16
F8 = ml_dtypes.float8_e4m3


def _build_nc(
    tiles: list[int] | None = None,
    bufs: int = 6,
    hq_tiles: tuple[int, int] = (2, 7),
    rank4_tile: int = 8,
    num_devices: int = NCORES,
) -> bass.Bass:
    nc = bacc.Bacc("TRN2", target_bir_lowering=False, debug=False, num_devices=num_devices)

    xt = nc.dram_tensor("xt", (P, KO, B), BF16, kind="ExternalInput")
    ht = nc.dram_tensor("ht", (P, KO, B), FP8, kind="ExternalInput")
    qk = nc.dram_tensor("qk", (P, KO, R), FP8, kind="ExternalInput")
    pt = nc.dram_tensor("pt", (R, JS), BF16, kind="ExternalInput")
    bm = nc.dram_tensor("bm", (P, KO, JS), BF16, kind="ExternalInput")
    hs = nc.dram_tensor("hs", (B, JS), FP8, kind="ExternalInput")
    ab = nc.dram_tensor("ab", (1, JS), F32, kind="ExternalInput")
    o = nc.dram_tensor("o", (B, JS), F32, kind="ExternalOutput")

    # b-tile sizes in k-chunks, end-tapered so that after the final DMA byte
    # lands only one half-chunk's matmul remains before the tail copies.  The
    # final chunk (KO-1) is streamed as two half-width (512-col) DMAs so each
    # PSUM bank's last matmul + copy chain starts as early as possible.
    TILES = tiles if tiles is not None else [4] * 14 + [2, 2, 1, 1, 1]
    assert sum(TILES) == KO - 1
    MAXKT = max(TILES)

    with TileContext(nc) as tc:
        with (
            tc.tile_pool(name="persist", bufs=1) as persist,
            tc.tile_pool(name="bpool", bufs=bufs) as bpool,
            tc.tile_pool(name="psum", bufs=1, space="PSUM") as psum_pool,
        ):
            xt_sb = persist.tile([P, KO, B], BF16)
            ht_sb = persist.tile([P, KO, B], FP8)
            qk_sb = persist.tile([P, KO, R], FP8)
            pt_sb = persist.tile([R, JS], BF16)
            hs_sb = persist.tile([B, JS], FP8)
            a1_sb = persist.tile([1, JS], F32)
            ab_sb = persist.tile([B, JS], F32)
            hqt_sb = persist.tile([R, B], BF16)
            o_sb = persist.tile([B, JS], F32)

            ps0 = psum_pool.tile([B, 512], F32)
            ps1 = psum_pool.tile([B, 512], F32)
            pshq = psum_pool.tile([R, B], F32)

            # Aux loads all go on the Activation queue; the SP queue leads
            # with the first b tile so the shared DMA engines never idle at
            # the head.  All transfers serialize on the shared DMA engines so
            # ordering does not change the total stream time; PE has ~2x
            # slack and catches up from any startup stall.
            nc.scalar.dma_start(out=xt_sb[:], in_=xt[:, :, :])
            nc.scalar.dma_start(out=hs_sb[:], in_=hs[:, :])
            nc.scalar.dma_start(out=a1_sb[:], in_=ab[:, :])
            nc.scalar.dma_start(out=qk_sb[:], in_=qk[:, :, :])
            nc.scalar.dma_start(out=ht_sb[:], in_=ht[:, :, :])
            nc.scalar.dma_start(out=pt_sb[:], in_=pt[:, :])

            # a_diag slice arrives as one row; broadcast to all 64 batch
            # partitions on the (otherwise idle) GPSIMD engine, then write
            # the diagonal term into PSUM as the accumulation seed.
            nc.gpsimd.partition_broadcast(ab_sb[:], a1_sb[:])
            nc.vector.tensor_mul(out=ps0[:], in0=hs_sb[:, 0:512], in1=ab_sb[:, 0:512])
            nc.vector.tensor_mul(out=ps1[:], in0=hs_sb[:, 512:JS], in1=ab_sb[:, 512:JS])

            hq_done = [0]

            def hq_emit(n):
                # hqT = q^T @ h^T: emit the next n k-chunks (fp8).
                for k in range(hq_done[0], min(hq_done[0] + n, KO)):
                    nc.tensor.matmul(
                        pshq[:],
                        qk_sb[:, k],
                        ht_sb[:, k],
                        start=(k == 0),
                        stop=(k == KO - 1),
                    )
                hq_done[0] = min(hq_done[0] + n, KO)

            # Main stream: x @ b_slice, single-pass bf16, accumulating onto
            # the pre-seeded PSUM banks.
            ko = 0
            for t, kt in enumerate(TILES):
                bfull = bpool.tile([P, MAXKT, JS], BF16, name="btile")
                btile = bfull[:, :kt]
                dma_eng = nc.sync if t % 2 == 0 else nc.scalar
                dma_eng.dma_start(out=btile[:], in_=bm[:, ko : ko + kt])
                for k4 in range(kt):
                    nc.tensor.matmul(
                        ps0[:], xt_sb[:, ko], btile[:, k4, 0:512], start=False, stop=False
                    )
                    nc.tensor.matmul(
                        ps1[:], xt_sb[:, ko], btile[:, k4, 512:JS], start=False, stop=False
                    )
                    ko += 1
                if hq_tiles[0] <= t < hq_tiles[1]:
                    # Spread the 64 hq matmuls over the window so they fill
                    # the PE's per-tile DMA-wait bubbles.
                    ng = hq_tiles[1] - hq_tiles[0]
                    hq_emit((KO + ng - 1) // ng)
                if t == rank4_tile:
                    hq_emit(KO)  # any remainder before the rank-4 term
                    # Rank-4 term accumulated straight into the main banks.
                    nc.vector.tensor_copy(out=hqt_sb[:], in_=pshq[:])
                    nc.tensor.matmul(
                        ps0[:], hqt_sb[:], pt_sb[:, 0:512], start=False, stop=False
                    )
                    nc.tensor.matmul(
                        ps1[:], hqt_sb[:], pt_sb[:, 512:JS], start=False, stop=False
                    )

            # Final chunk, streamed as a half then two quarters so each
            # PSUM bank's last matmul + copy chain starts as early as
            # possible and the very last matmul is only N=256 (~107 ns).
            assert ko == KO - 1
            bl_a = bpool.tile([P, MAXKT, JS], BF16, name="btile")
            nc.sync.dma_start(out=bl_a[:, 0, 0:512], in_=bm[:, KO - 1, 0:512])
            nc.tensor.matmul(
                ps0[:], xt_sb[:, KO - 1], bl_a[:, 0, 0:512], start=False, stop=True
            )
            nc.vector.tensor_copy(out=o_sb[:, 0:512], in_=ps0[:])
            nc.scalar.dma_start(out=bl_a[:, 0, 512:768], in_=bm[:, KO - 1, 512:768])
            nc.tensor.matmul(
                ps1[:, 0:256], xt_sb[:, KO - 1], bl_a[:, 0, 512:768],
                start=False, stop=True,
            )
            nc.sync.dma_start(out=bl_a[:, 0, 768:JS], in_=bm[:, KO - 1, 768:JS])
            nc.tensor.matmul(
                ps1[:, 256:512], xt_sb[:, KO - 1], bl_a[:, 0, 768:JS],
                start=False, stop=True,
            )
            nc.scalar.copy(out=o_sb[:, 512:JS], in_=ps1[:])
            # Single store: one HWDGE issue chain instead of two.
            nc.sync.dma_start(out=o[:, :], in_=o_sb[:, :])

    nc.finalize()
    return nc


_NC_CACHE = None


def _get_nc() -> bass.Bass:
    global _NC_CACHE
    if _NC_CACHE is None:
        _NC_CACHE = _build_nc()
    return _NC_CACHE


def _in_maps(h, x, a_diag, p_vec, q_vec, b_mat):
    # Replicated inputs, pre-permuted to k-on-partitions chunk layout.
    # xt[ki, ko, b] = x[b, ko*128 + ki]
    xt = np.ascontiguousarray(x.reshape(B, KO, P).transpose(2, 1, 0)).astype(BF)
    # h/q are only used for the tiny rank-4 term (~2.6% of output
    # magnitude), so fp8 e4m3 is plenty.  q values (~1.6e-2) sit in
    # e4m3 subnormal range, so scale q by 2^6 and p by 2^-6 (exact).
    ht = np.ascontiguousarray(h.reshape(B, KO, P).transpose(2, 1, 0)).astype(F8)
    # qk[ki, ko, r] = q_vec[ko*128 + ki, r]
    qk = np.ascontiguousarray(q_vec.reshape(KO, P, R).transpose(1, 0, 2) * 64.0).astype(F8)

    # b4[ko, ki, c, j] = b_mat[ko*128 + ki, c*1024 + j]
    b4 = b_mat.astype(BF).reshape(KO, P, NCORES, JS)

    in_maps = []
    for c in range(NCORES):
        j0 = c * JS
        bc = np.ascontiguousarray(b4[:, :, c, :].transpose(1, 0, 2))  # (P, KO, JS)
        in_maps.append(
            {
                "xt": xt,
                "ht": ht,
                "qk": qk,
                "pt": np.ascontiguousarray(p_vec[j0 : j0 + JS, :].T / 64.0).astype(BF),
                "bm": bc,
                "hs": np.ascontiguousarray(h[:, j0 : j0 + JS]).astype(F8),
                "ab": np.ascontiguousarray(a_diag[j0 : j0 + JS]).reshape(1, JS),
            }
        )
    return in_maps


def kernel(h, x, a_diag, p_vec, q_vec, b_mat) -> np.ndarray:
    h = np.ascontiguousarray(np.asarray(h, dtype=np.float32))
    x = np.ascontiguousarray(np.asarray(x, dtype=np.float32))
    a_diag = np.asarray(a_diag, dtype=np.float32)
    p_vec = np.asarray(p_vec, dtype=np.float32)
    q_vec = np.asarray(q_vec, dtype=np.float32)
    b_mat = np.asarray(b_mat, dtype=np.float32)

    nc = _get_nc()
    res = run_bass_kernel_spmd(
        nc, _in_maps(h, x, a_diag, p_vec, q_vec, b_mat), core_ids=list(range(NCORES))
    )
    return np.concatenate([r["o"] for r in res.results], axis=1)
